# revision 46
# baseline (speedup 1.0000x reference)
"""Trainium2 Bass kernel for nn_EnergyMapping (per-edge MLP -> energy sum).

Math (per molecule b):
    pre  = edge_embedding @ W1 + b1            # (E, H) with E = At*Nbr edges
    g    = softplus(pre)                        # shifted_softplus = g - log(2)
    y_e  = (g_e - log2) @ W2 + b2               # per-edge scalar
    E_b  = sum_e y_e
         = sum_h W2[h] * S[b,h] - E*log2*sum(W2) + E*b2,   S[b,h] = sum_e g[b,e,h]

Strategy: data-parallel over batch (16 molecules / 8 cores = 2 each), with the
input stream quantized to fp8e4 on host (4 MiB/core instead of 16 MiB fp32;
DMA floor drops from ~48us to ~12.6us). W1 stays bf16 as the stationary
matmul operand (mixed fp8 x bf16 is supported and halves the W1 quantization
error); end-to-end rel err ~2.2e-3 vs the 2e-2 gate, deterministic for the
fixed input seed. The toolchain has no softplus LUT (gen3 act tables), so
softplus costs two ACT passes -- the kernel makes the second one ~30x
narrower via a product tree:

Per core (ACT-bound: ~19us ACT busy; DMA 11.9, PE ~14, DVE ~17):
  - X^T shard [F=128, E=32768] fp8; W1 [128, 64] bf16 stationary. Matmuls
    column-tile pairs of 512-edge groups onto disjoint PSUM partition
    halves (two M=64 matmuls run concurrently); fp8 moving = 1 cyc/row.
    (DoubleRow fp8 was tried: dst partition 64 fails the ISA check, and
    without column tiling the elementwise grid doubles. A stride-0 k-tile
    broadcast hard-crashed the device.)
  - Chunk plans taper up for molecule 0 (first Exp at ~5us instead of ~9)
    and down for molecule 1 (short serial tail after the last DMA).
  - ACT pass 1: one wide Exp per chunk, PSUM [128, cols] -> SBUF bf16 t,
    bias=b1 free affine.
  - DVE: u = 1 + t (tensor_scalar, 4x bf16), then a 5-level tensor_tensor
    halves-multiply tree (2x bf16) reduces 32 edges to one product
    (tensor_reduce only runs 1x, the tree is ~40% cheaper; group products
    stay under ~4e18 on this data, far below bf16 max 3.4e38).
  - ACT pass 2: Ln with accum_out over the reduced products -> S[h] row
    sums (sum of softplus == ln of the grouped products). Two accumulator
    slots per molecule so the bulk Ln runs while the tail chunks stream.
  - Only acc [128, 4] leaves the device; the tiny W2 dot and the
    -E*log2*sum(W2) + E*b2 corrections run on host in fp64.
Timeline-sim (concourse.timeline_sim, tracks HW within ~1%): 30.2us; the
spread to the ~19us ACT floor is DMA spin-up (~2us), fill (~3us), the DVE
tree tail + Lns (~3us) and the exit drain barrier (~2us).
"""

import numpy as np

import concourse.bass as bass
import concourse.mybir as mybir
import concourse.tile as tile
from concourse import bacc
from concourse.bass_utils import run_bass_kernel_spmd

# Problem shapes (fixed by the task; kernel.py must be self-contained).
B, At, Nbr, F = 16, 256, 64, 128
H = F // 2                       # 64
N_CORES = 8
B_PER_CORE = B // N_CORES        # 2 molecules per core
EDGES_PER_MOL = At * Nbr         # 16384
E_PER_CORE = B_PER_CORE * EDGES_PER_MOL  # 32768

GROUP = 512                      # edges per matmul (one PSUM bank per half)
CHUNK = 4096                     # max edges per chunk
RED = 32                         # edges multiplied together before the Ln
# Chunk plans: molecule 0 tapers up so the first Exp/DVE work starts ~3us
# into the stream; molecule 1 tapers down so the serial post-DMA tail
# (tree + Ln of the final chunk) is short.
MOL_PLANS = [
    [1024, 1024, 2048, 4096, 4096, 4096],
    [4096, 4096, 4096, 2048, 2048],
]
# Which chunks run u=1+t on Pool instead of DVE: "none", "alt", "most".
# Pool offload measured WORSE in the timeline sim (+2us): the slow Pool
# pass (~2.9us vs 0.6us on DVE) adds un-hideable latency to each chunk's
# Exp -> tree chain even though it lowers DVE busy time.
ADD1_POOL = "none"
XBUFS = 6
SPLIT_LAST = False   # half-split of the final chunk's Exp/tree: no sim win
LN_TAIL_CHUNKS = 2   # chunks covered by the late (b) Ln slot per molecule
assert all(sum(p) == EDGES_PER_MOL for p in MOL_PLANS)
MOL_PCOLS = EDGES_PER_MOL // (2 * RED)   # reduced cols per molecule (256)

LOG2 = float(np.log(2.0))

_NC_CACHE = {}

# Both ACT passes (Exp, Ln) live in this single table set. The default
# table-load pass picks the first set containing each function, which would
# alternate table loads (~1.3us each) between the Exp and Ln instructions.
_ACT_SET_BOTH = "natural_log_exp_and_others"


class _EnergyBacc(bacc.Bacc):
    def insert_act_table_loads(self):
        import bass_rust as _bass_rust
        from concourse.hw_specs import get_activation_tables

        has_activation = any(
            isinstance(i, mybir.InstActivation)
            for b in self.main_func.blocks
            for i in b.instructions
        )
        if not has_activation:
            return
        all_tables = get_activation_tables(self.m.arch)
        if _ACT_SET_BOTH in all_tables:
            tables = [
                (name, funcs if name == _ACT_SET_BOTH else set())
                for name, funcs in all_tables.items()
            ]
        else:  # unexpected toolchain: fall back to default behaviour
            tables = list(all_tables.items())
        _bass_rust.insert_act_table_loads(self, tables)


def _build_nc(reps: int = 1, loop: int = 0, staggered: bool = False) -> bass.Bass:
    """Build the per-core Bass program. loop>0 wraps the body in a For_i
    hardware loop (slope-based HW timing only; output is just overwritten)."""
    from contextlib import ExitStack

    nc = _EnergyBacc("TRN2", target_bir_lowering=False, debug=False)
    f32 = mybir.dt.float32
    bf16 = mybir.dt.bfloat16
    fp8 = mybir.dt.float8e4

    # X stream: [F, E] with F on partitions; chunk slices are contiguous.
    xt = nc.dram_tensor("xt", [F, E_PER_CORE], fp8, kind="ExternalInput")
    w1 = nc.dram_tensor("w1", [F, H], bf16, kind="ExternalInput")
    b1c = nc.dram_tensor("b1c", [128, 1], f32, kind="ExternalInput")
    # Two accumulator slots per molecule (a: all but last chunk, b: last
    # chunk) so the bulk of each molecule's Ln runs before the final chunk.
    acc = nc.dram_tensor("acc", [128, 2 * B_PER_CORE], f32, kind="ExternalOutput")

    with tile.TileContext(nc) as tc:
        with ExitStack() as ctx:
            consts = ctx.enter_context(tc.tile_pool(name="consts", bufs=1))
            # Deep x prefetch: with only 3 bufs the chunk-3 DMA waits for
            # chunk-0's matmuls to free a buffer, starving ACT mid-stream.
            xpool = ctx.enter_context(tc.tile_pool(name="xpool", bufs=XBUFS))
            psum = ctx.enter_context(tc.tile_pool(name="psum", bufs=2, space="PSUM"))
            gpool = ctx.enter_context(tc.tile_pool(name="gpool", bufs=4))
            mpool = ctx.enter_context(tc.tile_pool(name="mpool", bufs=3))
            ppool = ctx.enter_context(tc.tile_pool(name="ppool", bufs=2))
            opool = ctx.enter_context(tc.tile_pool(name="opool", bufs=1))

            # w1/b1 go first on the same SP ring as the stream: they are tiny
            # (16 KiB) and the first matmul needs w1 before anything else.
            w1_sb = consts.tile([F, H], bf16)
            nc.sync.dma_start(w1_sb[:], w1[:, :])
            b1_sb = consts.tile([128, 1], f32)
            # ACT ring: keeps the tiny b1 load out of the x-stream's way.
            nc.scalar.dma_start(b1_sb[:], b1c[:, :])

            # Dummy first activation with no upstream deps: the act-table
            # load pass places the (~1.3us) LoadActFuncSet before the first
            # InstActivation, and the load inherits its queue position -- a
            # trivial activation here makes the table land at t~0 instead of
            # blocking behind chunk-0's matmuls.
            dummy = consts.tile([128, 1], f32)
            nc.vector.memset(dummy[:], 0.0)
            nc.scalar.activation(dummy[:], dummy[:],
                                 mybir.ActivationFunctionType.Exp,
                                 bias=0.0, scale=1.0)

            acc_sb = opool.tile([128, 2 * B_PER_CORE], f32)

            if loop:
                ctx.enter_context(tc.For_i(0, loop, 1, staggered_reset=staggered))

            for _rep in range(reps):
                # Zero-init: makes overwrite-vs-accumulate accum_out semantics
                # equivalent (each slot is written by exactly one instruction).
                nc.vector.memset(acc_sb[:], 0.0)

                e_base = 0   # edges consumed so far (dram offset = 2*e_base)
                for mol in range(B_PER_CORE):
                    plan = MOL_PLANS[mol % len(MOL_PLANS)]
                    p_mol = ppool.tile([128, MOL_PCOLS], bf16, tag="p")
                    p_off = 0
                    last_pcols = sum(plan[-LN_TAIL_CHUNKS:]) // (2 * RED)
                    split_off = MOL_PCOLS - last_pcols
                    for ci, csize in enumerate(plan):
                        cwide = csize // 2      # psum cols (2 edges per col)

                        xtile = xpool.tile([F, CHUNK], fp8, tag="xtile")
                        nc.sync.dma_start(
                            xtile[:, :csize],
                            xt[:, e_base : e_base + csize],
                        )

                        ps = psum.tile([128, CHUNK // 2], f32, tag="ps")
                        # Column-tiled pairs: the two M=64 matmuls land on
                        # disjoint PSUM partition halves and run concurrently
                        # in the PE array; each [64, 512] output fits one
                        # bank. Moving operand fp8 (1 cyc/row), stationary
                        # W1 bf16 (mixed dtypes are fine and halve the W1
                        # quantization error vs fp8 W1).
                        grp = min(GROUP, cwide)
                        for q in range(cwide // grp):
                            g0 = 2 * q * grp
                            nc.tensor.matmul(
                                ps[0:64, q * grp : (q + 1) * grp],
                                w1_sb[:], xtile[:, g0 : g0 + grp],
                                start=True, stop=True,
                            )
                            nc.tensor.matmul(
                                ps[64:128, q * grp : (q + 1) * grp],
                                w1_sb[:], xtile[:, g0 + grp : g0 + 2 * grp],
                                start=True, stop=True,
                            )
                        # The u = 1 + t pass alternates to the (otherwise
                        # idle) Pool engine for non-tail chunks: Pool is ~5x
                        # slower but off the DVE critical stream. Tail chunks
                        # stay on DVE to keep the serial tail short.
                        pool_add1 = (ci < len(plan) - 2) and (
                            (ADD1_POOL == "alt" and ci % 2 == 0)
                            or (ADD1_POOL == "most" and ci % 3 != 2)
                            or (ADD1_POOL == "early" and mol == 0 and ci <= 2))
                        add1_eng = nc.gpsimd if pool_add1 else nc.vector
                        # The final chunk's Exp/tree stage runs in two halves
                        # so its DVE tree starts ~1us before the Exp of the
                        # second half finishes (shortens the serial tail).
                        last = (SPLIT_LAST and mol == B_PER_CORE - 1
                                and ci == len(plan) - 1)
                        halves = ((0, cwide // 2), (cwide // 2, cwide)) if last \
                            else ((0, cwide),)
                        for h0, h1 in halves:
                            hw_ = h1 - h0
                            t = gpool.tile([128, CHUNK // 2], bf16, tag="t")
                            nc.scalar.activation(
                                t[:, h0:h1], ps[:, h0:h1],
                                mybir.ActivationFunctionType.Exp,
                                bias=b1_sb[:], scale=1.0,
                            )
                            u = gpool.tile([128, CHUNK // 2], bf16, tag="u")
                            add1_eng.tensor_scalar_add(
                                u[:, :hw_], t[:, h0:h1], 1.0)
                            # Halves-multiply tree: 32 edges -> 1 product.
                            m1 = mpool.tile([128, CHUNK // 4], bf16, tag="m1")
                            m2 = mpool.tile([128, CHUNK // 8], bf16, tag="m2")
                            src, dsts = u, [m1, m2, m1, m2]
                            w = hw_
                            level = 0
                            while w > 2 * (hw_ // RED):
                                w //= 2
                                dst = dsts[level]
                                nc.vector.tensor_tensor(
                                    dst[:, :w], src[:, 0:w], src[:, w : 2 * w],
                                    op=mybir.AluOpType.mult)
                                src = dst
                                level += 1
                            w //= 2
                            nc.vector.tensor_tensor(
                                p_mol[:, p_off : p_off + w],
                                src[:, 0:w], src[:, w : 2 * w],
                                op=mybir.AluOpType.mult)
                            p_off += w
                        e_base += csize

                        if ci == len(plan) - 1 - LN_TAIL_CHUNKS:
                            # Bulk Ln over everything but the last chunk's
                            # products, while the last chunk still streams.
                            lnout = gpool.tile([128, MOL_PCOLS], f32, tag="lnout")
                            nc.scalar.activation(
                                lnout[:, :split_off], p_mol[:, :split_off],
                                mybir.ActivationFunctionType.Ln,
                                bias=0.0, scale=1.0,
                                accum_out=acc_sb[:, 2 * mol : 2 * mol + 1],
                            )
                    lnout2 = gpool.tile([128, CHUNK // 32], f32, tag="lnout2")
                    nc.scalar.activation(
                        lnout2[:, :last_pcols], p_mol[:, split_off:],
                        mybir.ActivationFunctionType.Ln,
                        bias=0.0, scale=1.0,
                        accum_out=acc_sb[:, 2 * mol + 1 : 2 * mol + 2],
                    )
                    # Ship this molecule's slots as soon as its Lns are done
                    # (the last molecule's DMA is the only one on the tail).
                    # ACT ring: the SP ring would serialize loop iterations.
                    nc.scalar.dma_start(
                        acc[:, 2 * mol : 2 * mol + 2],
                        acc_sb[:, 2 * mol : 2 * mol + 2])
    nc.compile()
    return nc


def _get_nc() -> bass.Bass:
    if "main" not in _NC_CACHE:
        _NC_CACHE["main"] = _build_nc()
    return _NC_CACHE["main"]


def _make_in_maps(edge_embedding, W1, b1):
    import ml_dtypes

    e4 = ml_dtypes.float8_e4m3
    X = np.ascontiguousarray(edge_embedding, dtype=np.float32).reshape(
        B, EDGES_PER_MOL, F)
    w1 = np.ascontiguousarray(np.asarray(W1, np.float32).astype(ml_dtypes.bfloat16))
    b1c = np.concatenate([np.asarray(b1, np.float32)] * 2).reshape(128, 1)
    b1c = np.ascontiguousarray(b1c)
    in_maps = []
    for c in range(N_CORES):
        xc = X[c * B_PER_CORE : (c + 1) * B_PER_CORE].reshape(E_PER_CORE, F)
        # [F, E] shard, F on partitions, fp8e4 (trn E4M3 == ml_dtypes
        # float8_e4m3 incl. the +-240 max; |x| stays well under it here).
        xtc = np.ascontiguousarray(xc.T).astype(e4)
        in_maps.append({"xt": xtc, "w1": w1, "b1c": b1c})
    return in_maps


def _finalize(results, W1, b1, W2, b2):
    W2v = np.asarray(W2, np.float64).reshape(H)
    b2v = float(np.asarray(b2).reshape(()))
    out = np.empty((B, 1), np.float32)
    corr = -EDGES_PER_MOL * LOG2 * float(W2v.sum()) + EDGES_PER_MOL * b2v
    for c in range(N_CORES):
        acc = np.asarray(results[c]["acc"], np.float64)  # [128, 2*B_PER_CORE]
        S = acc[0:64, :] + acc[64:128, :]  # per-h softplus sums per slot
        for i in range(B_PER_CORE):
            b = c * B_PER_CORE + i
            Si = S[:, 2 * i] + S[:, 2 * i + 1]
            out[b, 0] = np.float32(Si @ W2v + corr)
    return out


def kernel_with_results(edge_embedding, W1, b1, W2, b2, trace=False, **run_kwargs):
    nc = _get_nc()
    in_maps = _make_in_maps(edge_embedding, W1, b1)
    core_ids = list(range(N_CORES))
    try:
        br = run_bass_kernel_spmd(nc, in_maps, core_ids, trace=trace, **run_kwargs)
    except ModuleNotFoundError:
        # Slim axon clients lack the NTFF profile hook (antenv.axon_hooks);
        # retry without tracing rather than failing the whole kernel.
        import os
        os.environ["BASS_NEVER_TRACE"] = "1"
        br = run_bass_kernel_spmd(nc, in_maps, core_ids, trace=False, **run_kwargs)
    out = _finalize(br.results, W1, b1, W2, b2)
    return out, br


def kernel(edge_embedding, W1, b1, W2, b2):
    out, _ = kernel_with_results(edge_embedding, W1, b1, W2, b2)
    return out


# revision 54
# speedup vs baseline: 1.0277x; 1.0277x over previous
"""Trainium2 Bass kernel for nn_EnergyMapping (per-edge MLP -> energy sum).

Math (per molecule b):
    pre  = edge_embedding @ W1 + b1            # (E, H) with E = At*Nbr edges
    g    = softplus(pre)                        # shifted_softplus = g - log(2)
    y_e  = (g_e - log2) @ W2 + b2               # per-edge scalar
    E_b  = sum_e y_e
         = sum_h W2[h] * S[b,h] - E*log2*sum(W2) + E*b2,   S[b,h] = sum_e g[b,e,h]

Strategy: data-parallel over batch (16 molecules / 8 cores = 2 each), with the
input stream quantized to fp8e4 on host (4 MiB/core instead of 16 MiB fp32;
DMA floor drops from ~48us to ~12.6us). W1 stays bf16 as the stationary
matmul operand (mixed fp8 x bf16 is supported and halves the W1 quantization
error); end-to-end rel err ~2.2e-3 vs the 2e-2 gate, deterministic for the
fixed input seed. The toolchain has no softplus LUT (gen3 act tables), so
softplus costs two ACT passes -- the kernel makes the second one ~30x
narrower via a product tree:

Per core (ACT-bound: ~19us ACT busy; DMA 11.9, PE ~14, DVE ~17):
  - X^T shard [F=128, E=32768] fp8; W1 [128, 64] bf16 stationary. Matmuls
    column-tile pairs of 512-edge groups onto disjoint PSUM partition
    halves (two M=64 matmuls run concurrently); fp8 moving = 1 cyc/row.
    (DoubleRow fp8 was tried: dst partition 64 fails the ISA check, and
    without column tiling the elementwise grid doubles. A stride-0 k-tile
    broadcast hard-crashed the device.)
  - Chunk plans taper up for molecule 0 (first Exp at ~5us instead of ~9)
    and down for molecule 1 (short serial tail after the last DMA).
  - ACT pass 1: one wide Exp per chunk, PSUM [128, cols] -> SBUF bf16 t,
    bias=b1 free affine.
  - DVE: u = 1 + t (tensor_scalar, 4x bf16), then a 5-level tensor_tensor
    halves-multiply tree (2x bf16) reduces 32 edges to one product
    (tensor_reduce only runs 1x, the tree is ~40% cheaper; group products
    stay under ~4e18 on this data, far below bf16 max 3.4e38).
  - ACT pass 2: Ln with accum_out over the reduced products -> S[h] row
    sums (sum of softplus == ln of the grouped products). Two accumulator
    slots per molecule so the bulk Ln runs while the tail chunks stream.
  - Only acc [128, 4] leaves the device; the tiny W2 dot and the
    -E*log2*sum(W2) + E*b2 corrections run on host in fp64.
Timeline-sim (concourse.timeline_sim, tracks HW within ~1%): 30.2us; the
spread to the ~19us ACT floor is DMA spin-up (~2us), fill (~3us), the DVE
tree tail + Lns (~3us) and the exit drain barrier (~2us).
"""

import numpy as np

import concourse.bass as bass
import concourse.mybir as mybir
import concourse.tile as tile
from concourse import bacc
from concourse.bass_utils import run_bass_kernel_spmd

# Problem shapes (fixed by the task; kernel.py must be self-contained).
B, At, Nbr, F = 16, 256, 64, 128
H = F // 2                       # 64
N_CORES = 8
B_PER_CORE = B // N_CORES        # 2 molecules per core
EDGES_PER_MOL = At * Nbr         # 16384
E_PER_CORE = B_PER_CORE * EDGES_PER_MOL  # 32768

GROUP = 512                      # edges per matmul (one PSUM bank per half)
CHUNK = 4096                     # max edges per chunk
RED = 32                         # edges multiplied together before the Ln
# Chunk plans: molecule 0 tapers up so the first Exp/DVE work starts ~3us
# into the stream; molecule 1 tapers down so the serial post-DMA tail
# (tree + Ln of the final chunk) is short.
MOL_PLANS = [
    [1024, 1024, 2048, 4096, 4096, 4096],
    [4096, 4096, 4096, 2048, 2048],
]
# Which chunks run u=1+t on Pool instead of DVE: "none", "alt", "most".
# Pool offload measured WORSE in the timeline sim (+2us): the slow Pool
# pass (~2.9us vs 0.6us on DVE) adds un-hideable latency to each chunk's
# Exp -> tree chain even though it lowers DVE busy time.
ADD1_POOL = "none"
XBUFS = 6
SPLIT_LAST = False   # half-split of the final chunk's Exp/tree: no sim win
LN_TAIL_CHUNKS = 2   # chunks covered by the late (b) Ln slot per molecule
assert all(sum(p) == EDGES_PER_MOL for p in MOL_PLANS)
MOL_PCOLS = EDGES_PER_MOL // (2 * RED)   # reduced cols per molecule (256)

LOG2 = float(np.log(2.0))

_NC_CACHE = {}

# Both ACT passes (Exp, Ln) live in this single table set. The default
# table-load pass picks the first set containing each function, which would
# alternate table loads (~1.3us each) between the Exp and Ln instructions.
_ACT_SET_BOTH = "natural_log_exp_and_others"


class _EnergyBacc(bacc.Bacc):
    def insert_act_table_loads(self):
        import bass_rust as _bass_rust
        from concourse.hw_specs import get_activation_tables

        has_activation = any(
            isinstance(i, mybir.InstActivation)
            for b in self.main_func.blocks
            for i in b.instructions
        )
        if not has_activation:
            return
        all_tables = get_activation_tables(self.m.arch)
        if _ACT_SET_BOTH in all_tables:
            tables = [
                (name, funcs if name == _ACT_SET_BOTH else set())
                for name, funcs in all_tables.items()
            ]
        else:  # unexpected toolchain: fall back to default behaviour
            tables = list(all_tables.items())
        _bass_rust.insert_act_table_loads(self, tables)


def _build_nc(reps: int = 1, loop: int = 0, staggered: bool = False,
              zero_b1: bool = True) -> bass.Bass:
    """Build the per-core Bass program. loop>0 wraps the body in a For_i
    hardware loop (slope-based HW timing only; output is just overwritten).
    zero_b1 skips the b1 DMA and uses a const-0 bias (b1 is zeros in this
    problem; the general path stays available)."""
    from contextlib import ExitStack

    nc = _EnergyBacc("TRN2", target_bir_lowering=False, debug=False)
    f32 = mybir.dt.float32
    bf16 = mybir.dt.bfloat16
    fp8 = mybir.dt.float8e4

    # X stream: [F, E] with F on partitions; chunk slices are contiguous.
    xt = nc.dram_tensor("xt", [F, E_PER_CORE], fp8, kind="ExternalInput")
    w1 = nc.dram_tensor("w1", [F, H], bf16, kind="ExternalInput")
    b1c = None if zero_b1 else nc.dram_tensor(
        "b1c", [128, 1], f32, kind="ExternalInput")
    # Two accumulator slots per molecule (a: all but last chunk, b: last
    # chunk) so the bulk of each molecule's Ln runs before the final chunk.
    acc = nc.dram_tensor("acc", [128, 2 * B_PER_CORE], f32, kind="ExternalOutput")

    with tile.TileContext(nc) as tc:
        with ExitStack() as ctx:
            consts = ctx.enter_context(tc.tile_pool(name="consts", bufs=1))
            # Deep x prefetch: with only 3 bufs the chunk-3 DMA waits for
            # chunk-0's matmuls to free a buffer, starving ACT mid-stream.
            xpool = ctx.enter_context(tc.tile_pool(name="xpool", bufs=XBUFS))
            psum = ctx.enter_context(tc.tile_pool(name="psum", bufs=2, space="PSUM"))
            gpool = ctx.enter_context(tc.tile_pool(name="gpool", bufs=4))
            mpool = ctx.enter_context(tc.tile_pool(name="mpool", bufs=3))
            ppool = ctx.enter_context(tc.tile_pool(name="ppool", bufs=2))
            opool = ctx.enter_context(tc.tile_pool(name="opool", bufs=1))

            # In the one-shot build, w1's DMA is deferred until after the
            # first chunk's dma_start: the HWDGE serializes descriptor
            # processing across rings, and every DMA pays a fixed
            # descriptor+engine-start+completion latency (~2.2us) -- putting
            # the big x0 transfer first lets its completion overlap w1's
            # descriptor phase (~0.9us off the first matmul). In loop mode
            # w1 must load outside the loop, so it goes first there.
            w1_sb = consts.tile([F, H], bf16)
            if loop:
                nc.sync.dma_start(w1_sb[:], w1[:, :])
            if zero_b1:
                b1_bias = 0.0
            else:
                b1_sb = consts.tile([128, 1], f32)
                # ACT ring: keeps the b1 load out of the x-stream's way.
                nc.scalar.dma_start(b1_sb[:], b1c[:, :])
                b1_bias = b1_sb[:]

            # Dummy first activation with no upstream deps: the act-table
            # load pass places the (~1.3us) LoadActFuncSet before the first
            # InstActivation, and the load inherits its queue position -- a
            # trivial activation here makes the table land at t~0 instead of
            # blocking behind chunk-0's matmuls.
            dummy = consts.tile([128, 1], f32)
            nc.vector.memset(dummy[:], 0.0)
            nc.scalar.activation(dummy[:], dummy[:],
                                 mybir.ActivationFunctionType.Exp,
                                 bias=0.0, scale=1.0)

            acc_sb = opool.tile([128, 2 * B_PER_CORE], f32)

            if loop:
                ctx.enter_context(tc.For_i(0, loop, 1, staggered_reset=staggered))

            for _rep in range(reps):
                # Zero-init: makes overwrite-vs-accumulate accum_out semantics
                # equivalent (each slot is written by exactly one instruction).
                nc.vector.memset(acc_sb[:], 0.0)

                e_base = 0   # edges consumed so far (dram offset = 2*e_base)
                for mol in range(B_PER_CORE):
                    plan = MOL_PLANS[mol % len(MOL_PLANS)]
                    # The final molecule's last chunk skips the DVE stage:
                    # a single Ln with the bias affine, ln(1*t + 1), computes
                    # softplus directly with accum_out, so nothing on the
                    # serial tail depends on DVE.
                    direct_tail = mol == B_PER_CORE - 1
                    ln_tail = 1 if direct_tail else LN_TAIL_CHUNKS
                    p_mol = ppool.tile([128, MOL_PCOLS], bf16, tag="p")
                    p_off = 0
                    last_pcols = sum(plan[-ln_tail:]) // (2 * RED)
                    split_off = MOL_PCOLS - last_pcols
                    for ci, csize in enumerate(plan):
                        cwide = csize // 2      # psum cols (2 edges per col)
                        direct = direct_tail and ci == len(plan) - 1

                        xtile = xpool.tile([F, CHUNK], fp8, tag="xtile")
                        nc.sync.dma_start(
                            xtile[:, :csize],
                            xt[:, e_base : e_base + csize],
                        )
                        if not loop and _rep == 0 and mol == 0 and ci == 0:
                            nc.sync.dma_start(w1_sb[:], w1[:, :])

                        ps = psum.tile([128, CHUNK // 2], f32, tag="ps")
                        # Column-tiled pairs: the two M=64 matmuls land on
                        # disjoint PSUM partition halves and run concurrently
                        # in the PE array; each [64, 512] output fits one
                        # bank. Moving operand fp8 (1 cyc/row), stationary
                        # W1 bf16 (mixed dtypes are fine and halve the W1
                        # quantization error vs fp8 W1).
                        grp = min(GROUP, cwide)
                        for q in range(cwide // grp):
                            g0 = 2 * q * grp
                            nc.tensor.matmul(
                                ps[0:64, q * grp : (q + 1) * grp],
                                w1_sb[:], xtile[:, g0 : g0 + grp],
                                start=True, stop=True,
                            )
                            nc.tensor.matmul(
                                ps[64:128, q * grp : (q + 1) * grp],
                                w1_sb[:], xtile[:, g0 + grp : g0 + 2 * grp],
                                start=True, stop=True,
                            )
                        # The u = 1 + t pass alternates to the (otherwise
                        # idle) Pool engine for non-tail chunks: Pool is ~5x
                        # slower but off the DVE critical stream. Tail chunks
                        # stay on DVE to keep the serial tail short.
                        pool_add1 = (ci < len(plan) - 2) and (
                            (ADD1_POOL == "alt" and ci % 2 == 0)
                            or (ADD1_POOL == "most" and ci % 3 != 2)
                            or (ADD1_POOL == "early" and mol == 0 and ci <= 2))
                        add1_eng = nc.gpsimd if pool_add1 else nc.vector
                        # The final chunk's Exp/tree stage runs in two halves
                        # so its DVE tree starts ~1us before the Exp of the
                        # second half finishes (shortens the serial tail).
                        last = (SPLIT_LAST and mol == B_PER_CORE - 1
                                and ci == len(plan) - 1)
                        halves = ((0, cwide // 2), (cwide // 2, cwide)) if last \
                            else ((0, cwide),)
                        for h0, h1 in halves:
                            hw_ = h1 - h0
                            t = gpool.tile([128, CHUNK // 2], bf16, tag="t")
                            nc.scalar.activation(
                                t[:, h0:h1], ps[:, h0:h1],
                                mybir.ActivationFunctionType.Exp,
                                bias=b1_bias, scale=1.0,
                            )
                            if direct:
                                lnd = gpool.tile([128, CHUNK // 2], f32,
                                                 tag="lnd")
                                nc.scalar.activation(
                                    lnd[:, :hw_], t[:, h0:h1],
                                    mybir.ActivationFunctionType.Ln,
                                    bias=1.0, scale=1.0,
                                    accum_out=acc_sb[:, 2 * mol + 1 :
                                                     2 * mol + 2],
                                )
                                continue
                            u = gpool.tile([128, CHUNK // 2], bf16, tag="u")
                            add1_eng.tensor_scalar_add(
                                u[:, :hw_], t[:, h0:h1], 1.0)
                            # Halves-multiply tree: 32 edges -> 1 product.
                            m1 = mpool.tile([128, CHUNK // 4], bf16, tag="m1")
                            m2 = mpool.tile([128, CHUNK // 8], bf16, tag="m2")
                            src, dsts = u, [m1, m2, m1, m2]
                            w = hw_
                            level = 0
                            while w > 2 * (hw_ // RED):
                                w //= 2
                                dst = dsts[level]
                                nc.vector.tensor_tensor(
                                    dst[:, :w], src[:, 0:w], src[:, w : 2 * w],
                                    op=mybir.AluOpType.mult)
                                src = dst
                                level += 1
                            w //= 2
                            nc.vector.tensor_tensor(
                                p_mol[:, p_off : p_off + w],
                                src[:, 0:w], src[:, w : 2 * w],
                                op=mybir.AluOpType.mult)
                            p_off += w
                        e_base += csize

                        if ci == len(plan) - 1 - ln_tail:
                            # Bulk Ln over everything but the tail chunks'
                            # products, while the tail still streams.
                            lnout = gpool.tile([128, MOL_PCOLS], f32, tag="lnout")
                            nc.scalar.activation(
                                lnout[:, :split_off], p_mol[:, :split_off],
                                mybir.ActivationFunctionType.Ln,
                                bias=0.0, scale=1.0,
                                accum_out=acc_sb[:, 2 * mol : 2 * mol + 1],
                            )
                    if not direct_tail:
                        lnout2 = gpool.tile([128, CHUNK // 32], f32, tag="lnout2")
                        nc.scalar.activation(
                            lnout2[:, :last_pcols], p_mol[:, split_off:],
                            mybir.ActivationFunctionType.Ln,
                            bias=0.0, scale=1.0,
                            accum_out=acc_sb[:, 2 * mol + 1 : 2 * mol + 2],
                        )
                    # Ship this molecule's slots as soon as its Lns are done
                    # (the last molecule's DMA is the only one on the tail).
                    # ACT ring: the SP ring would serialize loop iterations.
                    nc.scalar.dma_start(
                        acc[:, 2 * mol : 2 * mol + 2],
                        acc_sb[:, 2 * mol : 2 * mol + 2])
    nc.compile()
    return nc


def _get_nc(zero_b1: bool) -> bass.Bass:
    key = ("main", zero_b1)
    if key not in _NC_CACHE:
        _NC_CACHE[key] = _build_nc(zero_b1=zero_b1)
    return _NC_CACHE[key]


def _make_in_maps(edge_embedding, W1, b1):
    import ml_dtypes

    e4 = ml_dtypes.float8_e4m3
    zero_b1 = bool(np.all(np.asarray(b1) == 0))
    X = np.ascontiguousarray(edge_embedding, dtype=np.float32).reshape(
        B, EDGES_PER_MOL, F)
    w1 = np.ascontiguousarray(np.asarray(W1, np.float32).astype(ml_dtypes.bfloat16))
    b1c = np.concatenate([np.asarray(b1, np.float32)] * 2).reshape(128, 1)
    b1c = np.ascontiguousarray(b1c)
    in_maps = []
    for c in range(N_CORES):
        xc = X[c * B_PER_CORE : (c + 1) * B_PER_CORE].reshape(E_PER_CORE, F)
        # [F, E] shard, F on partitions, fp8e4 (trn E4M3 == ml_dtypes
        # float8_e4m3 incl. the +-240 max; |x| stays well under it here).
        xtc = np.ascontiguousarray(xc.T).astype(e4)
        m = {"xt": xtc, "w1": w1}
        if not zero_b1:
            m["b1c"] = b1c
        in_maps.append(m)
    return in_maps


def _finalize(results, W1, b1, W2, b2):
    W2v = np.asarray(W2, np.float64).reshape(H)
    b2v = float(np.asarray(b2).reshape(()))
    out = np.empty((B, 1), np.float32)
    corr = -EDGES_PER_MOL * LOG2 * float(W2v.sum()) + EDGES_PER_MOL * b2v
    for c in range(N_CORES):
        acc = np.asarray(results[c]["acc"], np.float64)  # [128, 2*B_PER_CORE]
        S = acc[0:64, :] + acc[64:128, :]  # per-h softplus sums per slot
        for i in range(B_PER_CORE):
            b = c * B_PER_CORE + i
            Si = S[:, 2 * i] + S[:, 2 * i + 1]
            out[b, 0] = np.float32(Si @ W2v + corr)
    return out


def kernel_with_results(edge_embedding, W1, b1, W2, b2, trace=False, **run_kwargs):
    nc = _get_nc(zero_b1=bool(np.all(np.asarray(b1) == 0)))
    in_maps = _make_in_maps(edge_embedding, W1, b1)
    core_ids = list(range(N_CORES))
    try:
        br = run_bass_kernel_spmd(nc, in_maps, core_ids, trace=trace, **run_kwargs)
    except ModuleNotFoundError:
        # Slim axon clients lack the NTFF profile hook (antenv.axon_hooks);
        # retry without tracing rather than failing the whole kernel.
        import os
        os.environ["BASS_NEVER_TRACE"] = "1"
        br = run_bass_kernel_spmd(nc, in_maps, core_ids, trace=False, **run_kwargs)
    out = _finalize(br.results, W1, b1, W2, b2)
    return out, br


def kernel(edge_embedding, W1, b1, W2, b2):
    out, _ = kernel_with_results(edge_embedding, W1, b1, W2, b2)
    return out
